# revision 35
# baseline (speedup 1.0000x reference)
"""Trainium2 Bass kernel for the DiCNN (WaveNet-like) module.

Sharding: pure data parallelism - 4 batch items per core on 8 cores.
On-chip layout: channels on partitions, time on the free dim; the four
batch items are stacked as 4x32-partition bands (block-diag weights),
2x64 bands for the 64-channel causal layer.

v2 design notes (vs the 152us baseline):
- Output is stored bf16 and upcast on host; the final-layer bias
  (b_sk2) is added host-side during the same dequant-style epilogue.
  This halves the dominant HBM store traffic (29.4 -> 14.7 MB/core).
- Stores are coalesced: each 512-wide time tile's [4 band, 512 t, 448
  oc] output goes out in ONE 1.83 MB DMA from a staged SBUF tile
  (einops-rearranged DRAM access pattern), instead of 16 x 229 KB.
- PSUM evacuation (the ACT/DVE bottleneck: ~1 elem/lane/cycle from
  PSUM) is done as one big strided op per band-tile (FD=1792) into the
  staging tile, split across the ACT and DVE engines.
- The skip path is accumulated in PSUM (ws0*g0 + ws1*g1 into one bank)
  and the w_sk1 layer is a plain diag4 matmul, so s1 is exactly
  [4 bands x 32 feat, T] with no bias/ones-row machinery.
- The gated-activation muls run on GpSimd, which is otherwise idle.
- x is pre-transposed on host and loaded with plain DMAs.
"""

import numpy as np
import ml_dtypes

import concourse.bacc as bacc
import concourse.tile as tile
from concourse import mybir
from concourse.bass_utils import run_bass_kernel_spmd

BF16 = mybir.dt.bfloat16
FP32 = mybir.dt.float32

B, T, C_IN, HID, C_OUT, K = 32, 4096, 64, 32, 448, 2
N_CORES = 8
BPC = B // N_CORES          # batches per core = 4
TT = 512                    # time-tile size
NT = T // TT                # 8 tiles
XCOLS = 4100                # 4097 padded
DELTA = 2                   # output-stage pipeline delay in tiles
N_WARMUP = 14               # dependency-free warm-up matmuls

AF = mybir.ActivationFunctionType
ALU = mybir.AluOpType

_cached_nc = None


def _f(x):
    return np.asarray(x, dtype=np.float32)


def _bf(x):
    return np.asarray(x, dtype=np.float32).astype(ml_dtypes.bfloat16)


def _tile4(v):
    return np.tile(_f(v).reshape(-1), 4).reshape(128, 1)


def diag4(w32):
    s = np.zeros((128, 128), np.float32)
    for i in range(4):
        s[32 * i:32 * i + 32, 32 * i:32 * i + 32] = w32
    return s


def prepare_weights(w_causal, b_causal, wd0, bd0, ws0, bs0, wo0, bo0,
                    wd1, bd1, ws1, bs1, wo1, bo1, w_sk1, b_sk1, w_sk2, b_sk2):
    """Host-side weight layout transforms (identical for every core)."""
    del wo1, bo1  # dead code: z after the last block is never used

    wc = np.zeros((128, 4, 128), np.float32)
    for p in range(2):
        for k in range(2):
            wcT = _f(w_causal)[:, :, k].T
            s = np.zeros((128, 128), np.float32)
            s[0:64, 64 * p:64 * p + 32] = wcT
            s[64:128, 64 * p + 32:64 * p + 64] = wcT
            wc[:, 2 * p + k, :] = s

    wd = np.zeros((128, 4, 128), np.float32)
    for blk, w in enumerate((wd0, wd1)):
        for k in range(2):
            wd[:, 2 * blk + k, :] = diag4(_f(w)[:, :, k].T)

    # skip/residual/head 1x1 convs, all as diag4 block-diagonal mats
    wsr = np.zeros((128, 4, 128), np.float32)
    wsr[:, 0, :] = diag4(_f(ws0)[:, :, 0].T)
    wsr[:, 1, :] = diag4(_f(wo0)[:, :, 0].T)
    wsr[:, 2, :] = diag4(_f(ws1)[:, :, 0].T)
    wsr[:, 3, :] = diag4(_f(w_sk1)[:, :, 0].T)

    # final conv weights, replicated on all four 32-partition bands
    w2r = np.zeros((128, 448), np.float32)
    w2T = _f(w_sk2)[:, :, 0].T
    for q in range(4):
        w2r[32 * q:32 * q + 32, :] = w2T

    bvecs = np.zeros((128, 6), np.float32)
    bvecs[:, 0] = _tile4(b_causal)[:, 0]
    bvecs[:, 1] = _tile4(bd0)[:, 0]
    bvecs[:, 2] = _tile4(bd1)[:, 0]
    bvecs[:, 3] = _tile4(bo0)[:, 0]
    bvecs[:, 4] = _tile4(_f(bs0) + _f(bs1))[:, 0]
    bvecs[:, 5] = _tile4(b_sk1)[:, 0]

    return dict(wc=_bf(wc), wd=_bf(wd), wsr=_bf(wsr), w2r=_bf(w2r),
                bvecs=np.ascontiguousarray(bvecs))


def prepare_x(x, core):
    """Per-core pre-transposed input staging array [2, 128, XCOLS] bf16.

    Column 0 is the causal zero pad (t=-1); column 1+t holds x[b, t, :]
    for the two batches of pair p stacked on the channel axis.
    """
    xT = np.zeros((2, 128, XCOLS), ml_dtypes.bfloat16)
    xb = _bf(x)
    for p in range(2):
        xT[p, 0:64, 1:1 + T] = xb[4 * core + 2 * p].T
        xT[p, 64:128, 1:1 + T] = xb[4 * core + 2 * p + 1].T
    return xT


def build_nc():
    nc = bacc.Bacc("TRN2", target_bir_lowering=False, debug=False,
                   num_devices=N_CORES)

    xT_d = nc.dram_tensor("xT", [2, 128, XCOLS], BF16, kind="ExternalInput")
    wc_d = nc.dram_tensor("wc", [128, 4, 128], BF16, kind="ExternalInput")
    wd_d = nc.dram_tensor("wd", [128, 4, 128], BF16, kind="ExternalInput")
    wsr_d = nc.dram_tensor("wsr", [128, 4, 128], BF16, kind="ExternalInput")
    w2_d = nc.dram_tensor("w2r", [128, 448], BF16, kind="ExternalInput")
    bv_d = nc.dram_tensor("bvecs", [128, 6], FP32, kind="ExternalInput")
    y_d = nc.dram_tensor("y", [BPC, T, C_OUT], BF16, kind="ExternalOutput")

    with tile.TileContext(nc) as tc:
        with (
            tc.tile_pool(name="const", bufs=1) as const,
            tc.tile_pool(name="persist", bufs=1) as persist,
            tc.tile_pool(name="act", bufs=3) as actp,
            tc.tile_pool(name="gtile", bufs=2) as gtp,
            tc.tile_pool(name="stg", bufs=2) as stgp,
            tc.tile_pool(name="pzB", bufs=1, space="PSUM") as pzbp,
            tc.tile_pool(name="pgg", bufs=1, space="PSUM") as pggp,
            tc.tile_pool(name="pks", bufs=2, space="PSUM") as pksp,
            tc.tile_pool(name="pout", bufs=2, space="PSUM") as poutp,
        ):
            # ---- constants (wc first - the warm-up burst needs it) ----
            wc_s = const.tile([128, 4, 128], BF16)
            nc.sync.dma_start(wc_s[:], wc_d.ap())
            # x chunk 0 next: it gates tile 0's causal conv
            x_s = [persist.tile([128, XCOLS], BF16, tag=f"x{p}",
                                name=f"x_s{p}") for p in range(2)]
            XCUTS = (0, 1028, 2052, 3076, XCOLS)
            for p in range(2):
                nc.sync.dma_start(x_s[p][:, 0:XCUTS[1]], xT_d[p, :, 0:XCUTS[1]])
            wd_s = const.tile([128, 4, 128], BF16)
            nc.sync.dma_start(wd_s[:], wd_d.ap())
            wsr_s = const.tile([128, 4, 128], BF16)
            nc.sync.dma_start(wsr_s[:], wsr_d.ap())
            w2_s = const.tile([128, 448], BF16)
            nc.sync.dma_start(w2_s[:], w2_d.ap())
            bv_s = const.tile([128, 6], FP32)
            nc.sync.dma_start(bv_s[:], bv_d.ap())

            bcausal = bv_s[:, 0:1]
            bd_v = (bv_s[:, 1:2], bv_s[:, 2:3])
            bo0_v = bv_s[:, 3:4]
            bskip_v = bv_s[:, 4:5]
            bsk1_v = bv_s[:, 5:6]

            # ---- persistent activations ----
            x_s = [persist.tile([128, XCOLS], BF16, tag=f"x{p}",
                                name=f"x_s{p}") for p in range(2)]
            for p in range(2):
                nc.sync.dma_start(x_s[p][:], xT_d[p])
            z0_s = persist.tile([128, 4100], BF16, tag="z0")
            nc.vector.memset(z0_s[:, 0:1], 0.0)
            z1_s = persist.tile([128, 4100], BF16, tag="z1")
            nc.vector.memset(z1_s[:, 0:2], 0.0)
            # s1 features: band q on partitions 32q..32q+32
            s1_s = persist.tile([128, T], BF16, tag="s1")

            # ---- PE warm-up burst (overlaps the input DMAs) ----
            wu_t = persist.tile([128, TT], BF16, tag="wu")
            nc.vector.memset(wu_t[:], 0.0)
            # dummy sigmoid first: steer the ACT-table pass to the
            # sigmoid_and_others set (contains tanh/relu/copy too) so only
            # one ACT_TABLE_LOAD is emitted instead of two
            scr_t = const.tile([128, 8], FP32)
            nc.vector.memset(scr_t[:], 0.0)
            nc.scalar.activation(scr_t[:], scr_t[:], AF.Sigmoid)
            hb_cnt = [0]

            def heartbeat(n, cols=TT):
                """Dependency-free PE filler matmuls: keep the HAM activity
                window busy so the 2.4 GHz clock state is reached/held."""
                for _ in range(n):
                    pwu = poutp.tile([128, 2, TT], FP32, tag="po",
                                     name=f"pwu_{hb_cnt[0]}")
                    hb_cnt[0] += 1
                    nc.tensor.matmul(pwu[:, hb_cnt[0] % 2, 0:cols],
                                     wc_s[:, 0, :], wu_t[:, 0:cols],
                                     start=True, stop=True)

            heartbeat(N_WARMUP)

            evac_cnt = [0]

            def emit_out_group(it, j, h, po):
                """Final matmuls for chunk j, band pair h of tile it: the two
                bands as a row-tiled pack (distinct row groups -> concurrent
                PE sub-arrays, LDWEIGHTS overlaps in-flight matmuls)."""
                tc0 = TT * it + 128 * j
                for b in (2 * h, 2 * h + 1):
                    nc.tensor.matmul(po[:, b - 2 * h, 0:448],
                                     s1_s[32 * b:32 * b + 32, tc0:tc0 + 128],
                                     w2_s[32 * b:32 * b + 32, :],
                                     start=True, stop=True,
                                     tile_position=(32 * b, 0))

            def emit_evac(it, j, h, po, stg):
                # strict alternation: consecutive pout slots must evacuate
                # on different engines so the slot ping-pong overlaps
                if evac_cnt[0] % 2 == 0:
                    nc.scalar.copy(stg[:, 2 * h:2 * h + 2, j, :],
                                   po[:, :, 0:448])
                else:
                    nc.vector.tensor_copy(stg[:, 2 * h:2 * h + 2, j, :],
                                          po[:, :, 0:448])
                evac_cnt[0] += 1

            def emit_stores(it, stg, qs):
                t0 = TT * it
                for q in qs:
                    yap = y_d[q, t0:t0 + TT, :].rearrange(
                        "(j p) c -> p j c", p=128)
                    nc.sync.dma_start(yap, stg[:, q])

            def out_stage(it, phase, state):
                """Emit one phase (0..7) of tile `it`'s output stage."""
                if it < 0:
                    if phase % 4 == 0:
                        heartbeat(1)
                    return
                if phase == 0:
                    state["stg"] = stgp.tile([128, 4, 4, 448], BF16,
                                             tag="stg", name=f"stg_{it}")
                j, h = phase >> 1, phase & 1
                po = poutp.tile([128, 2, TT], FP32, tag="po",
                                name=f"po_{it}_{phase}")
                emit_out_group(it, j, h, po)
                emit_evac(it, j, h, po, state["stg"])
                if phase == 7:
                    emit_stores(it, state["stg"])

            def emit_body(it, state):
                t0 = TT * it
                jt = it - DELTA
                # -- causal conv: 4 accumulating MMs -> z0
                # col-tiled: pair p writes partitions 64p..64p+64 only, so
                # the two pairs' matmuls run concurrently in the PE array
                pz = pzbp.tile([128, TT], FP32, tag="pzB", name=f"pz_{it}")
                for k in range(2):
                    for p in range(2):
                        rhs = x_s[p][:, t0 + k:t0 + k + TT]
                        nc.tensor.matmul(pz[64 * p:64 * p + 64, :],
                                         wc_s[:, 2 * p + k, 64 * p:64 * p + 64],
                                         rhs, start=(k == 0), stop=(k == 1),
                                         tile_position=(0, 64 * p))
                out_stage(jt, 0, state)
                nc.vector.tensor_scalar_add(z0_s[:, 1 + t0:1 + t0 + TT], pz[:],
                                            bcausal)

                # -- block 0: g0 = gate(conv(z0, wd0, dil=1))
                pg0 = pggp.tile([128, TT], FP32, tag="pgg", name=f"pg0_{it}")
                nc.tensor.matmul(pg0[:], wd_s[:, 0, :], z0_s[:, t0:t0 + TT],
                                 start=True, stop=False)
                nc.tensor.matmul(pg0[:], wd_s[:, 1, :],
                                 z0_s[:, 1 + t0:1 + t0 + TT],
                                 start=False, stop=True)
                out_stage(jt, 1, state)
                a0 = actp.tile([128, TT], BF16, tag="a", name=f"a0_{it}")
                nc.scalar.activation(a0[:], pg0[:], AF.Tanh, bias=bd_v[0])
                b0 = actp.tile([128, TT], BF16, tag="b", name=f"b0_{it}")
                nc.scalar.activation(b0[:], pg0[:], AF.Sigmoid, bias=bd_v[0])
                g0 = gtp.tile([128, TT], BF16, tag="g0", name=f"g0_{it}")
                nc.vector.tensor_mul(g0[:], a0[:], b0[:])
                out_stage(jt, 2, state)

                # -- skip accum (ws0*g0, later += ws1*g1) + residual z1
                psk = pksp.tile([128, TT], FP32, tag="pks", name=f"psk_{it}")
                nc.tensor.matmul(psk[:], wsr_s[:, 0, :], g0[:],
                                 start=True, stop=False)
                psB = pzbp.tile([128, TT], FP32, tag="pzB", name=f"psB_{it}")
                nc.tensor.matmul(psB[:], wsr_s[:, 1, :], g0[:],
                                 start=True, stop=True)
                out_stage(jt, 3, state)
                nc.vector.scalar_tensor_tensor(
                    z1_s[:, 2 + t0:2 + t0 + TT], psB[:], bo0_v,
                    z0_s[:, 1 + t0:1 + t0 + TT], ALU.add, ALU.add)

                # -- block 1: g1 = gate(conv(z1, wd1, dil=2))
                pg1 = pggp.tile([128, TT], FP32, tag="pgg", name=f"pg1_{it}")
                nc.tensor.matmul(pg1[:], wd_s[:, 2, :], z1_s[:, t0:t0 + TT],
                                 start=True, stop=False)
                nc.tensor.matmul(pg1[:], wd_s[:, 3, :],
                                 z1_s[:, 2 + t0:2 + t0 + TT],
                                 start=False, stop=True)
                out_stage(jt, 4, state)
                a1 = actp.tile([128, TT], BF16, tag="a", name=f"a1_{it}")
                nc.scalar.activation(a1[:], pg1[:], AF.Tanh, bias=bd_v[1])
                b1 = actp.tile([128, TT], BF16, tag="b", name=f"b1_{it}")
                nc.scalar.activation(b1[:], pg1[:], AF.Sigmoid, bias=bd_v[1])
                g1 = gtp.tile([128, TT], BF16, tag="g1", name=f"g1_{it}")
                nc.vector.tensor_mul(g1[:], a1[:], b1[:])
                out_stage(jt, 5, state)

                # -- head: s0 = relu(skip + bias), s1 = relu(wsk1@s0 + bias)
                nc.tensor.matmul(psk[:], wsr_s[:, 2, :], g1[:],
                                 start=False, stop=True)
                out_stage(jt, 6, state)
                s0_t = gtp.tile([128, TT], BF16, tag="s0", name=f"s0_{it}")
                nc.scalar.activation(s0_t[:], psk[:], AF.Relu, bias=bskip_v)
                ps5 = pksp.tile([128, TT], FP32, tag="pks", name=f"ps5_{it}")
                nc.tensor.matmul(ps5[:], wsr_s[:, 3, :], s0_t[:],
                                 start=True, stop=True)
                out_stage(jt, 7, state)
                nc.vector.tensor_scalar(s1_s[:, t0:t0 + TT], ps5[:],
                                        bsk1_v, 0.0, ALU.add, ALU.max)

            state = {}
            for it in range(NT):
                emit_body(it, state)
            for itt in range(NT - DELTA, NT):
                for phase in range(8):
                    out_stage(itt, phase, state)

    nc.compile()
    return nc


def get_nc():
    global _cached_nc
    if _cached_nc is None:
        _cached_nc = build_nc()
    return _cached_nc


def kernel(**inputs):
    nc = get_nc()
    w = prepare_weights(
        inputs["w_causal"], inputs["b_causal"],
        inputs["wd0"], inputs["bd0"], inputs["ws0"], inputs["bs0"],
        inputs["wo0"], inputs["bo0"],
        inputs["wd1"], inputs["bd1"], inputs["ws1"], inputs["bs1"],
        inputs["wo1"], inputs["bo1"],
        inputs["w_sk1"], inputs["b_sk1"], inputs["w_sk2"], inputs["b_sk2"])
    x = np.asarray(inputs["x"])
    in_maps = [{"xT": prepare_x(x, c), **w} for c in range(N_CORES)]
    res = run_bass_kernel_spmd(nc, in_maps, list(range(N_CORES)))
    out = np.concatenate(
        [np.asarray(res.results[c]["y"]) for c in range(N_CORES)], axis=0)
    out = out.astype(np.float32) + _f(inputs["b_sk2"])[None, None, :]
    return out


# revision 36
# speedup vs baseline: 1.0168x; 1.0168x over previous
"""Trainium2 Bass kernel for the DiCNN (WaveNet-like) module.

Sharding: pure data parallelism - 4 batch items per core on 8 cores.
On-chip layout: channels on partitions, time on the free dim; the four
batch items are stacked as 4x32-partition bands (block-diag weights),
2x64 bands for the 64-channel causal layer.

v2 design notes (vs the 152us baseline):
- Output is stored bf16 and upcast on host; the final-layer bias
  (b_sk2) is added host-side during the same dequant-style epilogue.
  This halves the dominant HBM store traffic (29.4 -> 14.7 MB/core).
- Stores are coalesced: each 512-wide time tile's [4 band, 512 t, 448
  oc] output goes out in ONE 1.83 MB DMA from a staged SBUF tile
  (einops-rearranged DRAM access pattern), instead of 16 x 229 KB.
- PSUM evacuation (the ACT/DVE bottleneck: ~1 elem/lane/cycle from
  PSUM) is done as one big strided op per band-tile (FD=1792) into the
  staging tile, split across the ACT and DVE engines.
- The skip path is accumulated in PSUM (ws0*g0 + ws1*g1 into one bank)
  and the w_sk1 layer is a plain diag4 matmul, so s1 is exactly
  [4 bands x 32 feat, T] with no bias/ones-row machinery.
- The gated-activation muls run on GpSimd, which is otherwise idle.
- x is pre-transposed on host and loaded with plain DMAs.
"""

import numpy as np
import ml_dtypes

import concourse.bacc as bacc
import concourse.tile as tile
from concourse import mybir
from concourse.bass_utils import run_bass_kernel_spmd

BF16 = mybir.dt.bfloat16
FP32 = mybir.dt.float32

B, T, C_IN, HID, C_OUT, K = 32, 4096, 64, 32, 448, 2
N_CORES = 8
BPC = B // N_CORES          # batches per core = 4
TT = 512                    # time-tile size
NT = T // TT                # 8 tiles
XCOLS = 4100                # 4097 padded
DELTA = 2                   # output-stage pipeline delay in tiles
N_WARMUP = 14               # dependency-free warm-up matmuls

AF = mybir.ActivationFunctionType
ALU = mybir.AluOpType

_cached_nc = None


def _f(x):
    return np.asarray(x, dtype=np.float32)


def _bf(x):
    return np.asarray(x, dtype=np.float32).astype(ml_dtypes.bfloat16)


def _tile4(v):
    return np.tile(_f(v).reshape(-1), 4).reshape(128, 1)


def diag4(w32):
    s = np.zeros((128, 128), np.float32)
    for i in range(4):
        s[32 * i:32 * i + 32, 32 * i:32 * i + 32] = w32
    return s


def prepare_weights(w_causal, b_causal, wd0, bd0, ws0, bs0, wo0, bo0,
                    wd1, bd1, ws1, bs1, wo1, bo1, w_sk1, b_sk1, w_sk2, b_sk2):
    """Host-side weight layout transforms (identical for every core)."""
    del wo1, bo1  # dead code: z after the last block is never used

    wc = np.zeros((128, 4, 128), np.float32)
    for p in range(2):
        for k in range(2):
            wcT = _f(w_causal)[:, :, k].T
            s = np.zeros((128, 128), np.float32)
            s[0:64, 64 * p:64 * p + 32] = wcT
            s[64:128, 64 * p + 32:64 * p + 64] = wcT
            wc[:, 2 * p + k, :] = s

    wd = np.zeros((128, 4, 128), np.float32)
    for blk, w in enumerate((wd0, wd1)):
        for k in range(2):
            wd[:, 2 * blk + k, :] = diag4(_f(w)[:, :, k].T)

    # skip/residual/head 1x1 convs, all as diag4 block-diagonal mats
    wsr = np.zeros((128, 4, 128), np.float32)
    wsr[:, 0, :] = diag4(_f(ws0)[:, :, 0].T)
    wsr[:, 1, :] = diag4(_f(wo0)[:, :, 0].T)
    wsr[:, 2, :] = diag4(_f(ws1)[:, :, 0].T)
    wsr[:, 3, :] = diag4(_f(w_sk1)[:, :, 0].T)

    # final conv weights, replicated on all four 32-partition bands
    w2r = np.zeros((128, 448), np.float32)
    w2T = _f(w_sk2)[:, :, 0].T
    for q in range(4):
        w2r[32 * q:32 * q + 32, :] = w2T

    bvecs = np.zeros((128, 6), np.float32)
    bvecs[:, 0] = _tile4(b_causal)[:, 0]
    bvecs[:, 1] = _tile4(bd0)[:, 0]
    bvecs[:, 2] = _tile4(bd1)[:, 0]
    bvecs[:, 3] = _tile4(bo0)[:, 0]
    bvecs[:, 4] = _tile4(_f(bs0) + _f(bs1))[:, 0]
    bvecs[:, 5] = _tile4(b_sk1)[:, 0]

    return dict(wc=_bf(wc), wd=_bf(wd), wsr=_bf(wsr), w2r=_bf(w2r),
                bvecs=np.ascontiguousarray(bvecs))


def prepare_x(x, core):
    """Per-core pre-transposed input staging array [2, 128, XCOLS] bf16.

    Column 0 is the causal zero pad (t=-1); column 1+t holds x[b, t, :]
    for the two batches of pair p stacked on the channel axis.
    """
    xT = np.zeros((2, 128, XCOLS), ml_dtypes.bfloat16)
    xb = _bf(x)
    for p in range(2):
        xT[p, 0:64, 1:1 + T] = xb[4 * core + 2 * p].T
        xT[p, 64:128, 1:1 + T] = xb[4 * core + 2 * p + 1].T
    return xT


def build_nc():
    nc = bacc.Bacc("TRN2", target_bir_lowering=False, debug=False,
                   num_devices=N_CORES)

    xT_d = nc.dram_tensor("xT", [2, 128, XCOLS], BF16, kind="ExternalInput")
    wc_d = nc.dram_tensor("wc", [128, 4, 128], BF16, kind="ExternalInput")
    wd_d = nc.dram_tensor("wd", [128, 4, 128], BF16, kind="ExternalInput")
    wsr_d = nc.dram_tensor("wsr", [128, 4, 128], BF16, kind="ExternalInput")
    w2_d = nc.dram_tensor("w2r", [128, 448], BF16, kind="ExternalInput")
    bv_d = nc.dram_tensor("bvecs", [128, 6], FP32, kind="ExternalInput")
    y_d = nc.dram_tensor("y", [BPC, T, C_OUT], BF16, kind="ExternalOutput")

    with tile.TileContext(nc) as tc:
        with (
            tc.tile_pool(name="const", bufs=1) as const,
            tc.tile_pool(name="persist", bufs=1) as persist,
            tc.tile_pool(name="act", bufs=3) as actp,
            tc.tile_pool(name="gtile", bufs=2) as gtp,
            tc.tile_pool(name="stg", bufs=2) as stgp,
            tc.tile_pool(name="pzB", bufs=1, space="PSUM") as pzbp,
            tc.tile_pool(name="pgg", bufs=1, space="PSUM") as pggp,
            tc.tile_pool(name="pks", bufs=2, space="PSUM") as pksp,
            tc.tile_pool(name="pout", bufs=2, space="PSUM") as poutp,
        ):
            # ---- constants (wc first - the warm-up burst needs it) ----
            wc_s = const.tile([128, 4, 128], BF16)
            nc.sync.dma_start(wc_s[:], wc_d.ap())
            # x chunk 0 next: it gates tile 0's causal conv
            x_s = [persist.tile([128, XCOLS], BF16, tag=f"x{p}",
                                name=f"x_s{p}") for p in range(2)]
            XCUTS = (0, 1028, 2052, 3076, XCOLS)
            for p in range(2):
                nc.sync.dma_start(x_s[p][:, 0:XCUTS[1]], xT_d[p, :, 0:XCUTS[1]])
            wd_s = const.tile([128, 4, 128], BF16)
            nc.sync.dma_start(wd_s[:], wd_d.ap())
            wsr_s = const.tile([128, 4, 128], BF16)
            nc.sync.dma_start(wsr_s[:], wsr_d.ap())
            w2_s = const.tile([128, 448], BF16)
            nc.sync.dma_start(w2_s[:], w2_d.ap())
            bv_s = const.tile([128, 6], FP32)
            nc.sync.dma_start(bv_s[:], bv_d.ap())

            bcausal = bv_s[:, 0:1]
            bd_v = (bv_s[:, 1:2], bv_s[:, 2:3])
            bo0_v = bv_s[:, 3:4]
            bskip_v = bv_s[:, 4:5]
            bsk1_v = bv_s[:, 5:6]

            # ---- persistent activations ----
            x_s = [persist.tile([128, XCOLS], BF16, tag=f"x{p}",
                                name=f"x_s{p}") for p in range(2)]
            for p in range(2):
                nc.sync.dma_start(x_s[p][:], xT_d[p])
            z0_s = persist.tile([128, 4100], BF16, tag="z0")
            nc.vector.memset(z0_s[:, 0:1], 0.0)
            z1_s = persist.tile([128, 4100], BF16, tag="z1")
            nc.vector.memset(z1_s[:, 0:2], 0.0)
            # s1 features: band q on partitions 32q..32q+32
            s1_s = persist.tile([128, T], BF16, tag="s1")

            # ---- PE warm-up burst (overlaps the input DMAs) ----
            wu_t = persist.tile([128, TT], BF16, tag="wu")
            nc.vector.memset(wu_t[:], 0.0)
            # dummy sigmoid first: steer the ACT-table pass to the
            # sigmoid_and_others set (contains tanh/relu/copy too) so only
            # one ACT_TABLE_LOAD is emitted instead of two
            scr_t = const.tile([128, 8], FP32)
            nc.vector.memset(scr_t[:], 0.0)
            nc.scalar.activation(scr_t[:], scr_t[:], AF.Sigmoid)
            hb_cnt = [0]

            def heartbeat(n, cols=TT):
                """Dependency-free PE filler matmuls: keep the HAM activity
                window busy so the 2.4 GHz clock state is reached/held."""
                for _ in range(n):
                    pwu = poutp.tile([128, 2, TT], FP32, tag="po",
                                     name=f"pwu_{hb_cnt[0]}")
                    hb_cnt[0] += 1
                    nc.tensor.matmul(pwu[:, hb_cnt[0] % 2, 0:cols],
                                     wc_s[:, 0, :], wu_t[:, 0:cols],
                                     start=True, stop=True)

            heartbeat(N_WARMUP)

            evac_cnt = [0]

            def emit_out_group(it, j, h, po):
                """Final matmuls for chunk j, band pair h of tile it: the two
                bands as a row-tiled pack (distinct row groups -> concurrent
                PE sub-arrays, LDWEIGHTS overlaps in-flight matmuls)."""
                tc0 = TT * it + 128 * j
                for b in (2 * h, 2 * h + 1):
                    nc.tensor.matmul(po[:, b - 2 * h, 0:448],
                                     s1_s[32 * b:32 * b + 32, tc0:tc0 + 128],
                                     w2_s[32 * b:32 * b + 32, :],
                                     start=True, stop=True,
                                     tile_position=(32 * b, 0))

            def emit_evac(it, j, h, po, stg):
                # strict alternation: consecutive pout slots must evacuate
                # on different engines so the slot ping-pong overlaps
                if evac_cnt[0] % 2 == 1:
                    nc.scalar.copy(stg[:, 2 * h:2 * h + 2, j, :],
                                   po[:, :, 0:448])
                else:
                    nc.vector.tensor_copy(stg[:, 2 * h:2 * h + 2, j, :],
                                          po[:, :, 0:448])
                evac_cnt[0] += 1

            def emit_stores(it, stg, qs):
                t0 = TT * it
                for q in qs:
                    yap = y_d[q, t0:t0 + TT, :].rearrange(
                        "(j p) c -> p j c", p=128)
                    nc.sync.dma_start(yap, stg[:, q])

            def out_stage(it, phase, state):
                """Emit one phase (0..7) of tile `it`'s output stage."""
                if it < 0:
                    if phase % 4 == 0:
                        heartbeat(1)
                    return
                if phase == 0:
                    state["stg"] = stgp.tile([128, 4, 4, 448], BF16,
                                             tag="stg", name=f"stg_{it}")
                j, h = phase >> 1, phase & 1
                po = poutp.tile([128, 2, TT], FP32, tag="po",
                                name=f"po_{it}_{phase}")
                emit_out_group(it, j, h, po)
                emit_evac(it, j, h, po, state["stg"])
                if phase == 7:
                    emit_stores(it, state["stg"])

            def emit_body(it, state):
                t0 = TT * it
                jt = it - DELTA
                # -- causal conv: 4 accumulating MMs -> z0
                # col-tiled: pair p writes partitions 64p..64p+64 only, so
                # the two pairs' matmuls run concurrently in the PE array
                pz = pzbp.tile([128, TT], FP32, tag="pzB", name=f"pz_{it}")
                for k in range(2):
                    for p in range(2):
                        rhs = x_s[p][:, t0 + k:t0 + k + TT]
                        nc.tensor.matmul(pz[64 * p:64 * p + 64, :],
                                         wc_s[:, 2 * p + k, 64 * p:64 * p + 64],
                                         rhs, start=(k == 0), stop=(k == 1),
                                         tile_position=(0, 64 * p))
                out_stage(jt, 0, state)
                nc.vector.tensor_scalar_add(z0_s[:, 1 + t0:1 + t0 + TT], pz[:],
                                            bcausal)

                # -- block 0: g0 = gate(conv(z0, wd0, dil=1))
                pg0 = pggp.tile([128, TT], FP32, tag="pgg", name=f"pg0_{it}")
                nc.tensor.matmul(pg0[:], wd_s[:, 0, :], z0_s[:, t0:t0 + TT],
                                 start=True, stop=False)
                nc.tensor.matmul(pg0[:], wd_s[:, 1, :],
                                 z0_s[:, 1 + t0:1 + t0 + TT],
                                 start=False, stop=True)
                out_stage(jt, 1, state)
                a0 = actp.tile([128, TT], BF16, tag="a", name=f"a0_{it}")
                nc.scalar.activation(a0[:], pg0[:], AF.Tanh, bias=bd_v[0])
                b0 = actp.tile([128, TT], BF16, tag="b", name=f"b0_{it}")
                nc.scalar.activation(b0[:], pg0[:], AF.Sigmoid, bias=bd_v[0])
                g0 = gtp.tile([128, TT], BF16, tag="g0", name=f"g0_{it}")
                nc.vector.tensor_mul(g0[:], a0[:], b0[:])
                out_stage(jt, 2, state)

                # -- skip accum (ws0*g0, later += ws1*g1) + residual z1
                psk = pksp.tile([128, TT], FP32, tag="pks", name=f"psk_{it}")
                nc.tensor.matmul(psk[:], wsr_s[:, 0, :], g0[:],
                                 start=True, stop=False)
                psB = pzbp.tile([128, TT], FP32, tag="pzB", name=f"psB_{it}")
                nc.tensor.matmul(psB[:], wsr_s[:, 1, :], g0[:],
                                 start=True, stop=True)
                out_stage(jt, 3, state)
                nc.vector.scalar_tensor_tensor(
                    z1_s[:, 2 + t0:2 + t0 + TT], psB[:], bo0_v,
                    z0_s[:, 1 + t0:1 + t0 + TT], ALU.add, ALU.add)

                # -- block 1: g1 = gate(conv(z1, wd1, dil=2))
                pg1 = pggp.tile([128, TT], FP32, tag="pgg", name=f"pg1_{it}")
                nc.tensor.matmul(pg1[:], wd_s[:, 2, :], z1_s[:, t0:t0 + TT],
                                 start=True, stop=False)
                nc.tensor.matmul(pg1[:], wd_s[:, 3, :],
                                 z1_s[:, 2 + t0:2 + t0 + TT],
                                 start=False, stop=True)
                out_stage(jt, 4, state)
                a1 = actp.tile([128, TT], BF16, tag="a", name=f"a1_{it}")
                nc.scalar.activation(a1[:], pg1[:], AF.Tanh, bias=bd_v[1])
                b1 = actp.tile([128, TT], BF16, tag="b", name=f"b1_{it}")
                nc.scalar.activation(b1[:], pg1[:], AF.Sigmoid, bias=bd_v[1])
                g1 = gtp.tile([128, TT], BF16, tag="g1", name=f"g1_{it}")
                nc.vector.tensor_mul(g1[:], a1[:], b1[:])
                out_stage(jt, 5, state)

                # -- head: s0 = relu(skip + bias), s1 = relu(wsk1@s0 + bias)
                nc.tensor.matmul(psk[:], wsr_s[:, 2, :], g1[:],
                                 start=False, stop=True)
                out_stage(jt, 6, state)
                s0_t = gtp.tile([128, TT], BF16, tag="s0", name=f"s0_{it}")
                nc.scalar.activation(s0_t[:], psk[:], AF.Relu, bias=bskip_v)
                ps5 = pksp.tile([128, TT], FP32, tag="pks", name=f"ps5_{it}")
                nc.tensor.matmul(ps5[:], wsr_s[:, 3, :], s0_t[:],
                                 start=True, stop=True)
                out_stage(jt, 7, state)
                nc.vector.tensor_scalar(s1_s[:, t0:t0 + TT], ps5[:],
                                        bsk1_v, 0.0, ALU.add, ALU.max)

            state = {}
            for it in range(NT):
                emit_body(it, state)
            for itt in range(NT - DELTA, NT):
                for phase in range(8):
                    out_stage(itt, phase, state)

    nc.compile()
    return nc


def get_nc():
    global _cached_nc
    if _cached_nc is None:
        _cached_nc = build_nc()
    return _cached_nc


def kernel(**inputs):
    nc = get_nc()
    w = prepare_weights(
        inputs["w_causal"], inputs["b_causal"],
        inputs["wd0"], inputs["bd0"], inputs["ws0"], inputs["bs0"],
        inputs["wo0"], inputs["bo0"],
        inputs["wd1"], inputs["bd1"], inputs["ws1"], inputs["bs1"],
        inputs["wo1"], inputs["bo1"],
        inputs["w_sk1"], inputs["b_sk1"], inputs["w_sk2"], inputs["b_sk2"])
    x = np.asarray(inputs["x"])
    in_maps = [{"xT": prepare_x(x, c), **w} for c in range(N_CORES)]
    res = run_bass_kernel_spmd(nc, in_maps, list(range(N_CORES)))
    out = np.concatenate(
        [np.asarray(res.results[c]["y"]) for c in range(N_CORES)], axis=0)
    out = out.astype(np.float32) + _f(inputs["b_sk2"])[None, None, :]
    return out
